# revision 14
# baseline (speedup 1.0000x reference)
"""Trainium2 Bass kernel for nn_Alpha_Free_GLM (gnn_message_passing).

Computation (per reference):
  x      = [S_e @ C_e.T | S_i @ C_i.T]                      (T, 40)
  y_b    = causal_conv(x, basis_b)  for b in 0..2           (T, 40) each
  syn_in[:, s] = sum_b w[s,b] y_b[:, s] + w[20+s,b] y_b[:, 20+s]
  tree recursion (binary tree, leaves->root) with tanh
  v      = tanh_root * exp(W0) + V_o

Key algebraic identity: every depthwise kernel is a linear combination of
SYN_BASIS_NO=3 shared basis filters, so the depthwise conv becomes 3 shared
Toeplitz matmuls over 40 channels + a per-channel weighted combine.

Sharding: time axis across 8 cores (12500 out-steps each + 256-row halo).
No collectives: each core's input slice (with halo) is prepared host-side.

Layout on device (per core):
  - S_pad (12800, 600) f32 time-major, 100 blocks of 128 rows.
  - per block: cast f32->bf16 (GPSIMD), PE-transpose 5 column-chunks
    (bf16, 1cyc/row), ACT copy PSUM->SBUF casting to fp8e4 (binary spikes
    are exact in fp8), then 5 accumulating matmuls against one-hot C_cat
    chunks (fp8) -> x block (128 t, 40 ch) -> bf16 in SBUF.
  - conv: out-block j needs in-blocks j, j+1, j+2; 9 matmuls
    (3 bases x 3 deltas) with constant bf16 Toeplitz stationaries.
  - combine on DVE with host-precomputed weight tiles; fold e/i halves
    into per-subunit layout syn_in[p, s*98 + j].
  - tree: per-subunit (128, 98) tiles; DVE scalar_tensor_tensor edges +
    ACT tanh with Theta fused as bias.
Output VOUT (128, 98) f32 per core; host de-interleaves t = j*128 + p.
"""

import sys
import numpy as np

for _p in ("/opt/trn_rl_repo",):
    if _p not in sys.path:
        sys.path.insert(0, _p)

import concourse.bass as bass
import concourse.bacc as bacc
import concourse.mybir as mybir
from concourse import tile
from concourse.bass_utils import run_bass_kernel_spmd

F32 = mybir.dt.float32
BF16 = mybir.dt.bfloat16
FP16 = mybir.dt.float16
FP8 = mybir.dt.float8e4
AF = mybir.ActivationFunctionType
ALU = mybir.AluOpType

SUB = 20
ENO = 500
INO = 100
TNO = 201
NBASIS = 3
NCH = 2 * SUB          # 40 channels: [e subunits | i subunits]
NSYN = ENO + INO       # 600
T_FULL = 100000
NCORES = 8
P = 128
HALO = 2 * P           # 256 >= TNO-1, and block aligned

FULL_CFG = dict(
    tc=12500,           # out timesteps per core
    nbi=100,            # in blocks  (12800 padded rows)
    nbo=98,             # out blocks (12544 >= 12500)
    sbc=4,              # conv superblock (out-blocks per PSUM residency)
)


def _np_dt(dt):
    return np.dtype(mybir.dt.np(dt))


def _tree_children(C_den):
    C = np.asarray(C_den)
    ch = {}
    for idx in range(SUB):
        ch[idx] = [int(c) for c in np.where(C[idx] == 1)[0]]
    return ch


def build_toeplitz(syn_basis):
    """T[b,d][p,i] = basis_b[i + 2*P - P*d - p] if 0<=idx<TNO else 0.

    y[t]=sum_tau k[tau] x[t-tau]; padded input row r maps to time r-HALO.
    Out-block j row i pulls from in-blocks j+d row p.
    """
    kern = np.asarray(syn_basis, np.float32)  # (3, 201)
    out = np.zeros((NBASIS, 3, P, P), np.float32)
    ii, pp = np.meshgrid(np.arange(P), np.arange(P), indexing="xy")
    # ii varies along axis1? build explicitly to avoid confusion:
    for b in range(NBASIS):
        for d in range(3):
            idx = np.arange(P)[None, :] + 2 * P - P * d - np.arange(P)[:, None]
            mask = (idx >= 0) & (idx < TNO)
            m = np.zeros((P, P), np.float32)
            m[mask] = kern[b][idx[mask]]
            out[b, d] = m
    return out  # (3,3,128,128) [p, i]


def build_program(cfg, consts):
    """consts: dict with toep (3,3,128,128) f32, ccat (600,40) f32,
    wtile (3,128,40*sbc) f32, expw (20,), theta (20,), vo float,
    children dict."""
    tc_, nbi, nbo, sbc = cfg["tc"], cfg["nbi"], cfg["nbo"], cfg["sbc"]
    nchunk = (NSYN + P - 1) // P  # 5
    ch_w = [min(P, NSYN - c * P) for c in range(nchunk)]  # 128,128,128,128,88

    nc = bacc.Bacc(None, target_bir_lowering=False, debug=False,
                   enable_partition_id=False)

    S = nc.dram_tensor("S", [nbi * P, NSYN], BF16, kind="ExternalInput")
    TOEP = nc.dram_tensor("TOEP", [NBASIS * 3 * P, P], FP16, kind="ExternalInput")
    CCAT = nc.dram_tensor("CCAT", [NSYN, NCH], FP8, kind="ExternalInput")
    WT = nc.dram_tensor("WT", [NBASIS * P, NCH * sbc], F32, kind="ExternalInput")
    IDEN = nc.dram_tensor("IDEN", [P, P], BF16, kind="ExternalInput")
    VOUT = nc.dram_tensor("VOUT", [P, nbo], F32, kind="ExternalOutput")

    expw = [float(v) for v in consts["expw"]]
    theta = [float(v) for v in consts["theta"]]
    vo = float(consts["vo"])
    children = consts["children"]

    with tile.TileContext(nc) as tc:
        with (
            tc.tile_pool(name="const", bufs=1) as cpool,
            tc.tile_pool(name="raw", bufs=nbi) as raw_pool,
            tc.tile_pool(name="stat", bufs=6) as stat_pool,
            tc.tile_pool(name="big", bufs=1) as big_pool,
            tc.tile_pool(name="cmb", bufs=4) as cmb_pool,
            tc.tile_pool(name="cols", bufs=SUB + 2) as col_pool,
            tc.tile_pool(name="ptr", bufs=2, space=bass.MemorySpace.PSUM) as ptr_pool,
            tc.tile_pool(name="ppr", bufs=2, space=bass.MemorySpace.PSUM) as ppr_pool,
            tc.tile_pool(name="pcv", bufs=2, space=bass.MemorySpace.PSUM) as pcv_pool,
        ):
            # ---- constants into SBUF ----
            toep_sb = cpool.tile([P, NBASIS * 3 * P], FP16, tag="toep")
            for k in range(NBASIS * 3):
                nc.sync.dma_start(
                    toep_sb[:, k * P:(k + 1) * P], TOEP[k * P:(k + 1) * P, :]
                )
            ccat_sb = cpool.tile([P, nchunk * NCH], FP8, tag="ccat")
            for c in range(nchunk):
                nc.sync.dma_start(
                    ccat_sb[: ch_w[c], c * NCH:(c + 1) * NCH],
                    CCAT[c * P: c * P + ch_w[c], :],
                )
            wt_sb = cpool.tile([P, NBASIS * NCH * sbc], F32, tag="wt")
            for b in range(NBASIS):
                nc.sync.dma_start(
                    wt_sb[:, b * NCH * sbc:(b + 1) * NCH * sbc],
                    WT[b * P:(b + 1) * P, :],
                )
            iden_sb = cpool.tile([P, P], BF16, tag="iden")
            nc.sync.dma_start(iden_sb[:], IDEN[:])

            synin = big_pool.tile([P, SUB * nbo], F32, tag="synin")
            # all projected blocks side by side: block j at cols [40j, 40j+40)
            xall = big_pool.tile([P, NCH * nbi], FP16, tag="xall")

            # ---- per in-block: load, cast, transpose, project ----
            for j in range(nbi):
                s_bf = raw_pool.tile([P, NSYN], BF16, tag="raw")
                nc.sync.dma_start(s_bf[:], S[j * P:(j + 1) * P, :])

                pp = ppr_pool.tile([P, NCH], F32, tag="ppr")
                # chunks 0..3 share two (128,256) psum tiles; chunk 4 separate
                for cpair in range(2):
                    tp = ptr_pool.tile([P, 2 * P], BF16, tag="ptr")
                    for half in range(2):
                        c = 2 * cpair + half
                        nc.tensor.transpose(
                            tp[: ch_w[c], half * P:(half + 1) * P],
                            s_bf[:, c * P: c * P + ch_w[c]],
                            iden_sb[:],
                        )
                    st = stat_pool.tile([P, 2 * P], FP8, tag="stat")
                    nc.scalar.copy(st[:], tp[:])
                    for half in range(2):
                        c = 2 * cpair + half
                        nc.tensor.matmul(
                            pp[:],
                            st[: ch_w[c], half * P:(half + 1) * P],
                            ccat_sb[: ch_w[c], c * NCH:(c + 1) * NCH],
                            start=(c == 0),
                            stop=False,
                        )
                c = 4
                tp = ptr_pool.tile([P, 2 * P], BF16, tag="ptr")
                nc.tensor.transpose(
                    tp[: ch_w[c], :P], s_bf[:, c * P: c * P + ch_w[c]], iden_sb[:]
                )
                st = stat_pool.tile([P, 2 * P], FP8, tag="stat")
                nc.scalar.copy(st[: ch_w[c], :P], tp[: ch_w[c], :P])
                nc.tensor.matmul(
                    pp[:],
                    st[: ch_w[c], :P],
                    ccat_sb[: ch_w[c], c * NCH:(c + 1) * NCH],
                    start=False,
                    stop=True,
                )
                nc.vector.tensor_copy(xall[:, j * NCH:(j + 1) * NCH], pp[:])

            # ---- conv + combine per superblock ----
            nsb = (nbo + sbc - 1) // sbc
            for k in range(nsb):
                j0 = k * sbc
                nblk = min(sbc, nbo - j0)
                w = NCH * nblk
                Y = pcv_pool.tile([P, NBASIS * NCH * sbc], F32, tag="pcv")
                # one matmul per (basis, delta) covers all nblk out-blocks:
                # out[:, c] = T_bd.T @ rhs[:, c] column-wise, and the rhs
                # columns for consecutive out-blocks are contiguous in xall.
                for b in range(NBASIS):
                    for d in range(3):
                        nc.tensor.matmul(
                            Y[:, b * NCH * sbc: b * NCH * sbc + w],
                            toep_sb[:, (b * 3 + d) * P:(b * 3 + d + 1) * P],
                            xall[:, (j0 + d) * NCH: (j0 + d) * NCH + w],
                            start=(d == 0),
                            stop=(d == 2),
                        )
                t0 = cmb_pool.tile([P, NCH * sbc], F32, tag="cmb0")
                t1 = cmb_pool.tile([P, NCH * sbc], F32, tag="cmb1")
                nc.vector.tensor_tensor(
                    t0[:, :w], Y[:, :w], wt_sb[:, :w], ALU.mult
                )
                nc.vector.tensor_tensor(
                    t1[:, :w], Y[:, NCH * sbc: NCH * sbc + w],
                    wt_sb[:, NCH * sbc: NCH * sbc + w], ALU.mult,
                )
                nc.vector.tensor_tensor(t0[:, :w], t0[:, :w], t1[:, :w], ALU.add)
                nc.vector.tensor_tensor(
                    t1[:, :w], Y[:, 2 * NCH * sbc: 2 * NCH * sbc + w],
                    wt_sb[:, 2 * NCH * sbc: 2 * NCH * sbc + w], ALU.mult,
                )
                nc.vector.tensor_tensor(t0[:, :w], t0[:, :w], t1[:, :w], ALU.add)
                # fold halves: synin[p, s*nbo + j0+jl] = t0[p, jl*40+s] + t0[p, jl*40+20+s]
                t0v = t0[:, :w].rearrange("p (j c) -> p c j", c=NCH)
                out_v = synin[:].rearrange("p (s j) -> p s j", j=nbo)[:, :, j0:j0 + nblk]
                nc.vector.tensor_tensor(
                    out_v, t0v[:, 0:SUB, :], t0v[:, SUB:NCH, :], ALU.add
                )

            # ---- tree recursion ----
            cols = [None] * SUB
            for s in range(SUB - 1, -1, -1):
                sin_s = synin[:, s * nbo:(s + 1) * nbo]
                kids = children[s]
                dst = col_pool.tile([P, nbo], F32, tag="cols")
                if not kids:
                    nc.scalar.activation(dst[:], sin_s, AF.Tanh, bias=theta[s])
                else:
                    acc = col_pool.tile([P, nbo], F32, tag="acc")
                    nc.vector.scalar_tensor_tensor(
                        acc[:], cols[kids[0]][:], expw[kids[0]], sin_s,
                        ALU.mult, ALU.add,
                    )
                    for c2 in kids[1:]:
                        nc.vector.scalar_tensor_tensor(
                            acc[:], cols[c2][:], expw[c2], acc[:],
                            ALU.mult, ALU.add,
                        )
                    nc.scalar.activation(dst[:], acc[:], AF.Tanh, bias=theta[s])
                cols[s] = dst

            vout_sb = col_pool.tile([P, nbo], F32, tag="vout")
            nc.scalar.activation(
                vout_sb[:], cols[0][:], AF.Copy, bias=vo, scale=expw[0]
            )
            nc.sync.dma_start(VOUT[:], vout_sb[:])

    return nc


def _prep_consts(C_syn_e, C_syn_i, syn_weights, syn_basis, W_sub, Theta, V_o,
                 C_den, sbc):
    ccat = np.zeros((NSYN, NCH), np.float32)
    ccat[:ENO, :SUB] = np.asarray(C_syn_e, np.float32).T
    ccat[ENO:, SUB:] = np.asarray(C_syn_i, np.float32).T
    toep = build_toeplitz(syn_basis)
    sw = np.asarray(syn_weights, np.float32)  # (40, 3)
    wtile = np.zeros((NBASIS, P, NCH * sbc), np.float32)
    for b in range(NBASIS):
        wtile[b, :, :] = np.tile(sw[:, b], sbc)[None, :]
    return dict(
        ccat=ccat,
        toep=toep,
        wtile=wtile,
        expw=np.exp(np.asarray(W_sub, np.float32)).astype(np.float32),
        theta=np.asarray(Theta, np.float32),
        vo=float(np.asarray(V_o).reshape(-1)[0]),
        children=_tree_children(C_den),
    )


def _input_maps(cfg, consts, S_e, S_i, n_cores, tc_):
    nbi = cfg["nbi"]
    sbc = cfg["sbc"]
    prows = nbi * P
    bf = _np_dt(BF16)
    f8 = _np_dt(FP8)
    toep_np = np.ascontiguousarray(
        consts["toep"].reshape(NBASIS * 3 * P, P)
    ).astype(np.float16)
    ccat_np = consts["ccat"].astype(f8)
    wt_np = np.ascontiguousarray(
        consts["wtile"].reshape(NBASIS * P, NCH * sbc)
    ).astype(np.float32)
    iden_np = np.eye(P, dtype=np.float32).astype(bf)

    S_e = np.asarray(S_e, np.float32)
    S_i = np.asarray(S_i, np.float32)
    t_full = S_e.shape[0]
    maps = []
    for m in range(n_cores):
        sp = np.zeros((prows, NSYN), bf)
        g0 = m * tc_ - HALO
        lo = max(0, g0)
        hi = min(t_full, g0 + prows)
        if hi > lo:
            sp[lo - g0: hi - g0, :ENO] = S_e[lo:hi].astype(bf)
            sp[lo - g0: hi - g0, ENO:] = S_i[lo:hi].astype(bf)
        maps.append({
            "S": sp, "TOEP": toep_np, "CCAT": ccat_np,
            "WT": wt_np, "IDEN": iden_np,
        })
    return maps


def kernel(S_e, S_i, C_syn_e, C_syn_i, syn_weights, syn_basis, W_sub, Theta,
           V_o, C_den, temp=None, test=None, _trace=False):
    cfg = dict(FULL_CFG)
    consts = _prep_consts(C_syn_e, C_syn_i, syn_weights, syn_basis, W_sub,
                          Theta, V_o, C_den, cfg["sbc"])
    nc = build_program(cfg, consts)
    nc.finalize()  # Bacc defers register allocation to compile/finalize
    in_maps = _input_maps(cfg, consts, S_e, S_i, NCORES, cfg["tc"])
    res = run_bass_kernel_spmd(nc, in_maps, list(range(NCORES)), trace=_trace)
    parts = []
    for m in range(NCORES):
        v = np.asarray(res.results[m]["VOUT"])  # (128, nbo)
        parts.append(v.T.reshape(-1)[: cfg["tc"]])
    final_voltage = np.concatenate(parts)[:T_FULL].astype(np.float32)

    sw = np.asarray(syn_weights, np.float32)
    sb = np.asarray(syn_basis, np.float32)
    out_filters = (sw @ sb).astype(np.float32)
    if _trace:
        return (final_voltage, out_filters,
                np.asarray(C_syn_e, np.float32),
                np.asarray(C_syn_i, np.float32)), res
    return (final_voltage, out_filters,
            np.asarray(C_syn_e, np.float32),
            np.asarray(C_syn_i, np.float32))
